# revision 7
# baseline (speedup 1.0000x reference)
"""Single-head GAT (DGL GATConv) forward on 8 Trainium2 NeuronCores.

Strategy (graph/data parallel, per the sharding hint):
  - Nodes padded 10000 -> 10240, sharded 1280/core (8 cores).
  - Phase 1 (per core): h_shard = feats_shard @ [W | W@attn_l] in one PE
    matmul pair (el appended as an extra output column); er for the shard
    is produced directly in ROW layout via lhsT = (W@attn_r).  Augmented
    rows Haug[n] = [h bf16 (256) | 1.0 | el_hi | el_lo] (260 bf16 = 520 B)
    staged to DRAM; AllGather -> full Haug [10240, 260].
  - Phase 2 (per core): edges pre-sorted by dst on host, grouped into 10
    windows of 128 dst nodes, each padded to EPW edges.  Per window:
      * 34x indirect_dma_start gathers (128 rows each) of Haug rows by src
        (the purpose-built dma_gather/dma_scatter_add ucode instructions
        crash this runtime; multi-row indirect DMA is also broken, so
        one-row-per-partition indirect DMA is the only working gather)
      * Sel[e,c,slot] = (iota == slot_e) built once per window in ONE
        broadcast-AP DVE op; er per edge = reduce_X(Sel * er_rep) where
        er_rep is er broadcast to all partitions via a K=1 matmul -- no
        er gather at all, er stays exact fp32
      * w = exp(leakyrelu(el + er)) batched over the window (DVE+ACT);
        SelW = Sel * w in one broadcast op
      * 34 PE matmuls accumulate psum[128 slots, 257] += SelW^T @ [h | 1]:
        unnormalized sums u and softmax denominator s in one pass (no
        max-subtraction needed: |logit| <= ~12)
      * epilogue: out = u / s + bias -> DMA to the output shard.  (s == 0
        only on padding slots whose rows the host discards.)

The environment executes ~40us/instruction (measured), so the kernel is
optimized for minimum instruction count, not bandwidth.

kernel(**inputs) takes full unsharded inputs, returns [10000, 256] fp32.
"""

import numpy as np
import ml_dtypes

N = 10000
E = 320000
D = 256
NPAD = 10240
NCORES = 8
SH = NPAD // NCORES          # 1280 nodes per core
WINN = 128                   # dst nodes per window
NW = SH // WINN              # 10 windows per core
EPW = 4352                   # padded edges per window (multiple of 128)
CH = EPW // 128              # 34 chunks of 128 edges per window
DA = 260                     # bf16 elements per augmented row (520 B)
NEG_SLOPE = 0.2
REPEAT = 1                   # whole-pipeline repeats (for differential timing)
ABLATE_GATHER = False        # timing ablation: skip indirect gathers
ABLATE_COLL = False          # timing ablation: skip the AllGather

_BF16 = ml_dtypes.bfloat16

_prog_cache = {}


def _prep_inputs(feats, W, attn_l, attn_r, bias, src, dst):
    """Host-side sharding/index prep. Returns in_maps (one dict per core)."""
    feats_pad = np.zeros((NPAD, D), np.float32)
    feats_pad[:N] = feats
    # fold attn vectors through W so el/er come straight from feats
    val = (W @ attn_l).astype(np.float32)
    var = (W @ attn_r).astype(np.float32)
    w_ext = np.concatenate([W.astype(np.float32), val[:, None]], axis=1)  # [256,257]

    order = np.argsort(dst, kind="stable")
    s_src = src[order].astype(np.int64)
    s_dst = dst[order].astype(np.int64)

    win = s_dst // WINN                       # global window id, 0..79
    nwin_g = NPAD // WINN                     # 80
    counts = np.bincount(win, minlength=nwin_g)
    assert counts.max() <= EPW, (counts.max(), EPW)
    starts = np.concatenate([[0], np.cumsum(counts)])

    src_idx = np.zeros((nwin_g, EPW), np.int32)
    slot = np.full((nwin_g, EPW), -1.0, np.float32)   # dst - window_base, -1 pad
    for g in range(nwin_g):
        a, b = starts[g], starts[g + 1]
        k = b - a
        src_idx[g, :k] = s_src[a:b]
        slot[g, :k] = s_dst[a:b] - g * WINN

    # per-chunk per-partition layout: token i -> [i % 128, i // 128]
    def tok(x):  # [nw, EPW] -> [nw, 128, CH]
        return np.ascontiguousarray(x.reshape(x.shape[0], CH, 128).transpose(0, 2, 1))

    comb = np.empty((nwin_g, 128, 2, CH), np.int32)
    comb[:, :, 0, :] = tok(src_idx)
    comb[:, :, 1, :] = tok(slot).view(np.int32)       # f32 bits in i32 array

    iota_row = np.broadcast_to(np.arange(128, dtype=np.float32), (128, 128))
    iota_row = np.ascontiguousarray(iota_row).astype(_BF16)

    in_maps = []
    for c in range(NCORES):
        featsT = np.ascontiguousarray(feats_pad[c * SH:(c + 1) * SH].T)  # [256, SH]
        lo, hi = c * NW, (c + 1) * NW
        in_maps.append({
            "featsT": featsT,
            "Wext": w_ext,
            "var_in": np.ascontiguousarray(var[:, None]),
            "bias_in": np.ascontiguousarray(bias.astype(np.float32)[None, :]),
            "iota_row": iota_row,
            "comb": np.ascontiguousarray(comb[lo:hi]),
        })
    return in_maps


def _build_program(ncores):
    import concourse.bass as bass
    import concourse.tile as tile
    from concourse import bacc, mybir
    from contextlib import ExitStack

    f32 = mybir.dt.float32
    bf16 = mybir.dt.bfloat16
    i32 = mybir.dt.int32

    nc = bacc.Bacc(
        "TRN2", target_bir_lowering=False, debug=False, num_devices=ncores
    )

    featsT = nc.dram_tensor("featsT", [D, SH], f32, kind="ExternalInput").ap()
    Wext = nc.dram_tensor("Wext", [D, D + 1], f32, kind="ExternalInput").ap()
    var_in = nc.dram_tensor("var_in", [D, 1], f32, kind="ExternalInput").ap()
    bias_in = nc.dram_tensor("bias_in", [1, D], f32, kind="ExternalInput").ap()
    iota_in = nc.dram_tensor("iota_row", [128, 128], bf16, kind="ExternalInput").ap()
    combw = nc.dram_tensor("comb", [NW, 128, 2, CH], i32, kind="ExternalInput").ap()
    out_ext = nc.dram_tensor("out", [SH, D], f32, kind="ExternalOutput").ap()

    hstage = nc.dram_tensor("hstage", [SH, DA], bf16).ap()
    if ncores > 1:
        hfull = nc.dram_tensor("hfull", [NPAD, DA], bf16, addr_space="Shared").ap()
    else:
        hfull = hstage

    NT = SH // 128  # node tiles per core

    with tile.TileContext(nc) as tc, ExitStack() as ctx:
        const = ctx.enter_context(tc.tile_pool(name="const", bufs=1))

        w_sb = const.tile([128, 2, D + 1], f32, tag="w_sb")   # Wext (2 k-halves)
        nc.sync.dma_start(w_sb[:, 0, :], Wext[0:128, :])
        nc.sync.dma_start(w_sb[:, 1, :], Wext[128:256, :])
        var_sb = const.tile([128, 2, 1], f32, tag="var_sb")
        nc.sync.dma_start(var_sb[:, 0, :], var_in[0:128, :])
        nc.sync.dma_start(var_sb[:, 1, :], var_in[128:256, :])
        iota_sb = const.tile([128, 128], bf16, tag="iota")
        nc.sync.dma_start(iota_sb[:], iota_in[:])
        bias_row = const.tile([1, D], f32, tag="bias_row")
        nc.sync.dma_start(bias_row[:], bias_in[:])
        ones_col = const.tile([1, 128], f32, tag="ones_col")
        nc.vector.memset(ones_col[:], 1.0)
        er_rows = const.tile([1, SH], f32, tag="er_rows")

        bias_rep = const.tile([128, D], f32, tag="bias_rep")
        with tc.tile_pool(name="psum_b", bufs=1, space="PSUM") as psb:
            pb = psb.tile([128, D], f32)
            nc.tensor.matmul(pb[:], lhsT=ones_col[:], rhs=bias_row[:],
                             start=True, stop=True)
            nc.vector.tensor_copy(bias_rep[:], pb[:])

        for _rep in range(REPEAT):
            # ------------- Phase 1: h, el, er for the local shard ----------
            with tc.tile_pool(name="p1_sbuf", bufs=2) as p1, \
                 tc.tile_pool(name="p1_big", bufs=1) as p1b, \
                 tc.tile_pool(name="p1_psum", bufs=2, space="PSUM") as pp:
                ftT = p1b.tile([128, 2, SH], f32, tag="ftT")
                nc.sync.dma_start(ftT[:, 0, :], featsT[0:128, :])
                nc.sync.dma_start(ftT[:, 1, :], featsT[128:256, :])

                for nt in range(NT):
                    nsl = bass.ts(nt, 128)
                    hp = pp.tile([128, D + 1], f32, tag="hp")
                    erp = pp.tile([1, 128], f32, tag="erp")
                    for k in range(2):
                        nc.tensor.matmul(hp[:], lhsT=ftT[:, k, nsl],
                                         rhs=w_sb[:, k, :],
                                         start=(k == 0), stop=(k == 1))
                    for k in range(2):
                        nc.tensor.matmul(erp[:], lhsT=var_sb[:, k, :],
                                         rhs=ftT[:, k, nsl],
                                         start=(k == 0), stop=(k == 1))
                    nc.vector.tensor_copy(er_rows[:, nsl], erp[:])

                    haug = p1.tile([128, DA], bf16, tag="haug")
                    nc.vector.tensor_copy(haug[:, 0:D], hp[:, 0:D])
                    nc.vector.memset(haug[:, D:D + 1], 1.0)
                    # el as hi/lo bf16 pair (cols 257, 258): el ~= hi + lo
                    nc.vector.tensor_copy(haug[:, 257:258], hp[:, D:D + 1])
                    lo32 = p1.tile([128, 1], f32, tag="lo32")
                    nc.vector.tensor_tensor(out=lo32[:], in0=hp[:, D:D + 1],
                                            in1=haug[:, 257:258],
                                            op=mybir.AluOpType.subtract)
                    nc.vector.tensor_copy(haug[:, 258:259], lo32[:])
                    nc.vector.memset(haug[:, 259:260], 0.0)
                    nc.sync.dma_start(hstage[nt * 128:(nt + 1) * 128, :], haug[:])

            if ncores > 1 and not ABLATE_COLL:
                nc.gpsimd.collective_compute(
                    "AllGather",
                    mybir.AluOpType.bypass,
                    replica_groups=[list(range(ncores))],
                    ins=[hstage[:]],
                    outs=[hfull[:]],
                )

            # ------------- Phase 2: per-window attention + aggregation -----
            with tc.tile_pool(name="gidx", bufs=2) as gidx_p, \
                 tc.tile_pool(name="ghr", bufs=2) as ghr_p, \
                 tc.tile_pool(name="sel", bufs=2) as sel_p, \
                 tc.tile_pool(name="erpr", bufs=2) as erpr_p, \
                 tc.tile_pool(name="small", bufs=2) as small_p, \
                 tc.tile_pool(name="selw", bufs=2) as selw_p, \
                 tc.tile_pool(name="outp", bufs=2) as out_p, \
                 tc.tile_pool(name="psum_u", bufs=2, space="PSUM") as psu_p, \
                 tc.tile_pool(name="psum_r", bufs=2, space="PSUM") as psr_p:
                for w in range(NW):
                    comb = gidx_p.tile([128, 2, CH], i32, tag="comb")
                    nc.sync.dma_start(comb[:], combw[w])
                    six = comb[:, 0, :]
                    slot_f = comb[:, 1, :].bitcast(f32)

                    ghr = ghr_p.tile([128, CH, DA], bf16, tag="ghr")
                    if ABLATE_GATHER:
                        nc.vector.memset(ghr[:], 0.5)
                    else:
                        for c in range(CH):
                            nc.gpsimd.indirect_dma_start(
                                out=ghr[:, c, :], out_offset=None, in_=hfull[:],
                                in_offset=bass.IndirectOffsetOnAxis(
                                    ap=six[:, c:c + 1], axis=0))

                    # er replicated to all partitions via K=1 matmul
                    err_ps = psr_p.tile([128, 128], f32, tag="err_ps")
                    nc.tensor.matmul(err_ps[:], lhsT=ones_col[:],
                                     rhs=er_rows[:, bass.ts(w, 128)],
                                     start=True, stop=True)
                    er_rep = erpr_p.tile([128, 128], f32, tag="er_rep")
                    nc.vector.tensor_copy(er_rep[:], err_ps[:])

                    # Sel for ALL chunks in one broadcast-AP op
                    sel = sel_p.tile([128, CH, 128], bf16, tag="sel")
                    nc.vector.tensor_tensor(
                        out=sel[:],
                        in0=iota_sb[:, None, :].broadcast_to([128, CH, 128]),
                        in1=slot_f[:, :, None].broadcast_to([128, CH, 128]),
                        op=mybir.AluOpType.is_equal,
                    )
                    # er per edge = reduce_X(Sel * er_rep)  (exact fp32)
                    er_prod = erpr_p.tile([128, CH, 128], f32, tag="er_prod")
                    nc.vector.tensor_tensor(
                        out=er_prod[:],
                        in0=sel[:],
                        in1=er_rep[:, None, :].broadcast_to([128, CH, 128]),
                        op=mybir.AluOpType.mult,
                    )
                    t_sb = small_p.tile([128, CH], f32, tag="t")
                    nc.vector.tensor_reduce(
                        out=t_sb[:], in_=er_prod[:],
                        axis=mybir.AxisListType.X, op=mybir.AluOpType.add,
                    )
                    # t += el_hi + el_lo  (gathered bf16 cols 257, 258)
                    nc.vector.tensor_tensor(
                        out=t_sb[:, :, None], in0=t_sb[:, :, None],
                        in1=ghr[:, :, 257:258], op=mybir.AluOpType.add,
                    )
                    nc.vector.tensor_tensor(
                        out=t_sb[:, :, None], in0=t_sb[:, :, None],
                        in1=ghr[:, :, 258:259], op=mybir.AluOpType.add,
                    )
                    t2_sb = small_p.tile([128, CH], f32, tag="t2")
                    nc.vector.tensor_scalar_mul(t2_sb[:], t_sb[:], NEG_SLOPE)
                    lk_sb = small_p.tile([128, CH], f32, tag="lk")
                    nc.vector.tensor_tensor(
                        out=lk_sb[:], in0=t_sb[:], in1=t2_sb[:],
                        op=mybir.AluOpType.max,
                    )
                    wv_sb = small_p.tile([128, CH], f32, tag="wv")
                    nc.scalar.activation(
                        wv_sb[:], lk_sb[:], mybir.ActivationFunctionType.Exp
                    )

                    selw = selw_p.tile([128, CH, 128], bf16, tag="selw")
                    nc.vector.tensor_tensor(
                        out=selw[:],
                        in0=sel[:],
                        in1=wv_sb[:, :, None].broadcast_to([128, CH, 128]),
                        op=mybir.AluOpType.mult,
                    )
                    psum_u = psu_p.tile([128, D + 1], f32, tag="pu")
                    for c in range(CH):
                        nc.tensor.matmul(
                            psum_u[:], lhsT=selw[:, c, :], rhs=ghr[:, c, 0:D + 1],
                            start=(c == 0), stop=(c == CH - 1),
                        )

                    # out = u / s + bias  (s==0 only on discarded pad slots)
                    rcp = small_p.tile([128, 1], f32, tag="rcp")
                    nc.vector.reciprocal(rcp[:], psum_u[:, D:D + 1])
                    ot = out_p.tile([128, D], f32, tag="ot")
                    nc.vector.tensor_scalar_mul(ot[:], psum_u[:, 0:D], rcp[:])
                    nc.vector.tensor_tensor(
                        out=ot[:], in0=ot[:], in1=bias_rep[:],
                        op=mybir.AluOpType.add,
                    )
                    nc.sync.dma_start(out_ext[w * 128:(w + 1) * 128, :], ot[:])

    nc.compile()
    return nc


def _get_program(ncores):
    if ncores not in _prog_cache:
        _prog_cache[ncores] = _build_program(ncores)
    return _prog_cache[ncores]


def kernel(feats, W, attn_l, attn_r, bias, src, dst):
    from concourse.bass_utils import run_bass_kernel_spmd

    feats = np.asarray(feats, np.float32)
    W = np.asarray(W, np.float32)
    attn_l = np.asarray(attn_l, np.float32)
    attn_r = np.asarray(attn_r, np.float32)
    bias = np.asarray(bias, np.float32)
    src = np.asarray(src)
    dst = np.asarray(dst)

    in_maps = _prep_inputs(feats, W, attn_l, attn_r, bias, src, dst)
    nc = _get_program(NCORES)
    res = run_bass_kernel_spmd(nc, in_maps, list(range(NCORES)))
    shards = [np.asarray(res.results[c]["out"]) for c in range(NCORES)]
    return np.concatenate(shards, axis=0)[:N].astype(np.float32)


# revision 9
# speedup vs baseline: 1.1380x; 1.1380x over previous
"""Single-head GAT (DGL GATConv) forward on 8 Trainium2 NeuronCores.

Strategy (graph/data parallel, per the sharding hint):
  - Nodes padded 10000 -> 10240, sharded 1280/core (8 cores).
  - Phase 1 (per core): h_shard = feats_shard @ [W | W@attn_l] in one PE
    matmul pair (el appended as an extra output column); er for the shard
    is produced directly in ROW layout via lhsT = (W@attn_r).  Augmented
    rows Haug[n] = [h bf16 (256) | 1.0 | el_hi | el_lo] (260 bf16 = 520 B)
    staged to DRAM; AllGather -> full Haug [10240, 260].
  - Phase 2 (per core): edges pre-sorted by dst on host, grouped into 10
    windows of 128 dst nodes, each padded to EPW edges.  Per window:
      * 34x indirect_dma_start gathers (128 rows each) of Haug rows by src
        (the purpose-built dma_gather/dma_scatter_add ucode instructions
        crash this runtime; multi-row indirect DMA is also broken, so
        one-row-per-partition indirect DMA is the only working gather)
      * Sel[e,c,slot] = (iota == slot_e) built once per window in ONE
        broadcast-AP DVE op; er per edge = reduce_X(Sel * er_rep) where
        er_rep is er broadcast to all partitions via a K=1 matmul -- no
        er gather at all, er stays exact fp32
      * w = exp(leakyrelu(el + er)) batched over the window (DVE+ACT);
        SelW = Sel * w in one broadcast op
      * 34 PE matmuls accumulate psum[128 slots, 257] += SelW^T @ [h | 1]:
        unnormalized sums u and softmax denominator s in one pass (no
        max-subtraction needed: |logit| <= ~12)
      * epilogue: out = u / s + bias -> DMA to the output shard.  (s == 0
        only on padding slots whose rows the host discards.)

The environment executes ~40us/instruction (measured), so the kernel is
optimized for minimum instruction count, not bandwidth.

kernel(**inputs) takes full unsharded inputs, returns [10000, 256] fp32.
"""

import numpy as np
import ml_dtypes

N = 10000
E = 320000
D = 256
NPAD = 10240
NCORES = 8
SH = NPAD // NCORES          # 1280 nodes per core
WINN = 128                   # dst nodes per window
NW = SH // WINN              # 10 windows per core
EPW = 4352                   # padded edges per window (multiple of 128)
CH = EPW // 128              # 34 chunks of 128 edges per window
DA = 260                     # bf16 elements per augmented row (520 B)
NEG_SLOPE = 0.2
REPEAT = 1                   # whole-pipeline repeats (for differential timing)
ABLATE_GATHER = False        # timing ablation: skip indirect gathers
ABLATE_COLL = False          # timing ablation: skip the AllGather

_BF16 = ml_dtypes.bfloat16

_prog_cache = {}


def _prep_inputs(feats, W, attn_l, attn_r, bias, src, dst):
    """Host-side sharding/index prep. Returns in_maps (one dict per core)."""
    feats_pad = np.zeros((NPAD, D), np.float32)
    feats_pad[:N] = feats
    # fold attn vectors through W so el/er come straight from feats
    val = (W @ attn_l).astype(np.float32)
    var = (W @ attn_r).astype(np.float32)
    w_ext = np.concatenate([W.astype(np.float32), val[:, None]], axis=1)  # [256,257]

    order = np.argsort(dst, kind="stable")
    s_src = src[order].astype(np.int64)
    s_dst = dst[order].astype(np.int64)

    win = s_dst // WINN                       # global window id, 0..79
    nwin_g = NPAD // WINN                     # 80
    counts = np.bincount(win, minlength=nwin_g)
    assert counts.max() <= EPW, (counts.max(), EPW)
    starts = np.concatenate([[0], np.cumsum(counts)])

    src_idx = np.zeros((nwin_g, EPW), np.int32)
    slot = np.full((nwin_g, EPW), -1.0, np.float32)   # dst - window_base, -1 pad
    for g in range(nwin_g):
        a, b = starts[g], starts[g + 1]
        k = b - a
        src_idx[g, :k] = s_src[a:b]
        slot[g, :k] = s_dst[a:b] - g * WINN

    # per-chunk per-partition layout: token i -> [i % 128, i // 128]
    def tok(x):  # [nw, EPW] -> [nw, 128, CH]
        return np.ascontiguousarray(x.reshape(x.shape[0], CH, 128).transpose(0, 2, 1))

    comb = np.empty((nwin_g, 128, 2, CH), np.int32)
    comb[:, :, 0, :] = tok(src_idx)
    comb[:, :, 1, :] = tok(slot).view(np.int32)       # f32 bits in i32 array

    iota_row = np.broadcast_to(np.arange(128, dtype=np.float32), (128, 128))
    iota_row = np.ascontiguousarray(iota_row).astype(_BF16)

    in_maps = []
    for c in range(NCORES):
        featsT = np.ascontiguousarray(feats_pad[c * SH:(c + 1) * SH].T)  # [256, SH]
        lo, hi = c * NW, (c + 1) * NW
        in_maps.append({
            "featsT": featsT,
            "Wext": w_ext,
            "var_in": np.ascontiguousarray(var[:, None]),
            "bias_in": np.ascontiguousarray(bias.astype(np.float32)[None, :]),
            "iota_row": iota_row,
            "comb": np.ascontiguousarray(comb[lo:hi]),
        })
    return in_maps


def _build_program(ncores):
    import concourse.bass as bass
    import concourse.tile as tile
    from concourse import bacc, mybir
    from contextlib import ExitStack

    f32 = mybir.dt.float32
    bf16 = mybir.dt.bfloat16
    i32 = mybir.dt.int32

    nc = bacc.Bacc(
        "TRN2", target_bir_lowering=False, debug=False, num_devices=ncores
    )

    featsT = nc.dram_tensor("featsT", [D, SH], f32, kind="ExternalInput").ap()
    Wext = nc.dram_tensor("Wext", [D, D + 1], f32, kind="ExternalInput").ap()
    var_in = nc.dram_tensor("var_in", [D, 1], f32, kind="ExternalInput").ap()
    bias_in = nc.dram_tensor("bias_in", [1, D], f32, kind="ExternalInput").ap()
    iota_in = nc.dram_tensor("iota_row", [128, 128], bf16, kind="ExternalInput").ap()
    combw = nc.dram_tensor("comb", [NW, 128, 2, CH], i32, kind="ExternalInput").ap()
    out_ext = nc.dram_tensor("out", [SH, D], f32, kind="ExternalOutput").ap()

    hstage = nc.dram_tensor("hstage", [SH, DA], bf16).ap()
    if ncores > 1:
        hfull = nc.dram_tensor("hfull", [NPAD, DA], bf16, addr_space="Shared").ap()
    else:
        hfull = hstage

    NT = SH // 128  # node tiles per core

    with tile.TileContext(nc) as tc, ExitStack() as ctx:
        const = ctx.enter_context(tc.tile_pool(name="const", bufs=1))

        w_sb = const.tile([128, 2, D + 1], f32, tag="w_sb")   # Wext (2 k-halves)
        nc.sync.dma_start(w_sb[:, 0, :], Wext[0:128, :])
        nc.sync.dma_start(w_sb[:, 1, :], Wext[128:256, :])
        var_sb = const.tile([128, 2, 1], f32, tag="var_sb")
        nc.sync.dma_start(var_sb[:, 0, :], var_in[0:128, :])
        nc.sync.dma_start(var_sb[:, 1, :], var_in[128:256, :])
        iota_sb = const.tile([128, 128], bf16, tag="iota")
        nc.sync.dma_start(iota_sb[:], iota_in[:])
        bias_row = const.tile([1, D], f32, tag="bias_row")
        nc.sync.dma_start(bias_row[:], bias_in[:])
        ones_col = const.tile([1, 128], f32, tag="ones_col")
        nc.vector.memset(ones_col[:], 1.0)
        er_rows = const.tile([1, SH], f32, tag="er_rows")

        bias_rep = const.tile([128, D], f32, tag="bias_rep")
        with tc.tile_pool(name="psum_b", bufs=1, space="PSUM") as psb:
            pb = psb.tile([128, D], f32)
            nc.tensor.matmul(pb[:], lhsT=ones_col[:], rhs=bias_row[:],
                             start=True, stop=True)
            nc.vector.tensor_copy(bias_rep[:], pb[:])

        for _rep in range(REPEAT):
            # ------------- Phase 1: h, el, er for the local shard ----------
            with tc.tile_pool(name="p1_sbuf", bufs=2) as p1, \
                 tc.tile_pool(name="p1_big", bufs=1) as p1b, \
                 tc.tile_pool(name="p1_psum", bufs=2, space="PSUM") as pp:
                ftT = p1b.tile([128, 2, SH], f32, tag="ftT")
                nc.sync.dma_start(ftT[:, 0, :], featsT[0:128, :])
                nc.sync.dma_start(ftT[:, 1, :], featsT[128:256, :])

                hbig = p1b.tile([128, NT, DA], bf16, tag="hbig")
                nc.vector.memset(hbig[:, :, D:D + 1], 1.0)
                nc.vector.memset(hbig[:, :, 259:260], 0.0)
                for nt in range(NT):
                    nsl = bass.ts(nt, 128)
                    hp = pp.tile([128, D + 1], f32, tag="hp")
                    erp = pp.tile([1, 128], f32, tag="erp")
                    for k in range(2):
                        nc.tensor.matmul(hp[:], lhsT=ftT[:, k, nsl],
                                         rhs=w_sb[:, k, :],
                                         start=(k == 0), stop=(k == 1))
                    for k in range(2):
                        nc.tensor.matmul(erp[:], lhsT=var_sb[:, k, :],
                                         rhs=ftT[:, k, nsl],
                                         start=(k == 0), stop=(k == 1))
                    nc.vector.tensor_copy(er_rows[:, nsl], erp[:])

                    nc.vector.tensor_copy(hbig[:, nt, 0:D], hp[:, 0:D])
                    # el as hi/lo bf16 pair (cols 257, 258): el ~= hi + lo
                    nc.vector.tensor_copy(hbig[:, nt, 257:258], hp[:, D:D + 1])
                    lo32 = p1.tile([128, 1], f32, tag="lo32")
                    nc.vector.tensor_tensor(out=lo32[:], in0=hp[:, D:D + 1],
                                            in1=hbig[:, nt, 257:258],
                                            op=mybir.AluOpType.subtract)
                    nc.vector.tensor_copy(hbig[:, nt, 258:259], lo32[:])
                # single staging DMA: hbig [p, nt, :] -> hstage [nt*128+p, :]
                nc.sync.dma_start(
                    hstage.rearrange("(t p) d -> p t d", p=128), hbig[:])

            if ncores > 1 and not ABLATE_COLL:
                nc.gpsimd.collective_compute(
                    "AllGather",
                    mybir.AluOpType.bypass,
                    replica_groups=[list(range(ncores))],
                    ins=[hstage[:]],
                    outs=[hfull[:]],
                )

            # ------------- Phase 2: per-window attention + aggregation -----
            with tc.tile_pool(name="gidx", bufs=2) as gidx_p, \
                 tc.tile_pool(name="ghr", bufs=3) as ghr_p, \
                 tc.tile_pool(name="sel", bufs=3) as sel_p, \
                 tc.tile_pool(name="erpr", bufs=1) as erpr_p, \
                 tc.tile_pool(name="erprod", bufs=2) as erprod_p, \
                 tc.tile_pool(name="small", bufs=2) as small_p, \
                 tc.tile_pool(name="selw", bufs=2) as selw_p, \
                 tc.tile_pool(name="outp", bufs=2) as out_p, \
                 tc.tile_pool(name="psum_u", bufs=2, space="PSUM") as psu_p, \
                 tc.tile_pool(name="psum_r", bufs=2, space="PSUM") as psr_p:
                # er replicated to all partitions for ALL windows (K=1 matmuls)
                er_rep = erpr_p.tile([128, NW, 128], f32, tag="er_rep")
                for j in range(NW // 2):
                    err_ps = psr_p.tile([128, 2, 128], f32, tag="err_ps")
                    nc.tensor.matmul(err_ps[:], lhsT=ones_col[:],
                                     rhs=er_rows[:, j * 256:(j + 1) * 256],
                                     start=True, stop=True)
                    nc.vector.tensor_copy(er_rep[:, 2 * j:2 * j + 2, :], err_ps[:])
                for w in range(NW):
                    comb = gidx_p.tile([128, 2, CH], i32, tag="comb")
                    nc.sync.dma_start(comb[:], combw[w])
                    six = comb[:, 0, :]
                    slot_f = comb[:, 1, :].bitcast(f32)

                    ghr = ghr_p.tile([128, CH, DA], bf16, tag="ghr")
                    if ABLATE_GATHER:
                        nc.vector.memset(ghr[:], 0.5)
                    else:
                        for c in range(CH):
                            nc.gpsimd.indirect_dma_start(
                                out=ghr[:, c, :], out_offset=None, in_=hfull[:],
                                in_offset=bass.IndirectOffsetOnAxis(
                                    ap=six[:, c:c + 1], axis=0))

                    # Sel for ALL chunks in one broadcast-AP op
                    sel = sel_p.tile([128, CH, 128], bf16, tag="sel")
                    nc.vector.tensor_tensor(
                        out=sel[:],
                        in0=iota_sb[:, None, :].broadcast_to([128, CH, 128]),
                        in1=slot_f[:, :, None].broadcast_to([128, CH, 128]),
                        op=mybir.AluOpType.is_equal,
                    )
                    # er per edge = reduce_X(Sel * er_rep)  (exact fp32)
                    er_prod = erprod_p.tile([128, CH, 128], f32, tag="er_prod")
                    nc.vector.tensor_tensor(
                        out=er_prod[:],
                        in0=sel[:],
                        in1=er_rep[:, w, None, :].broadcast_to([128, CH, 128]),
                        op=mybir.AluOpType.mult,
                    )
                    t_sb = small_p.tile([128, CH], f32, tag="t")
                    nc.vector.tensor_reduce(
                        out=t_sb[:], in_=er_prod[:],
                        axis=mybir.AxisListType.X, op=mybir.AluOpType.add,
                    )
                    # t += el_hi + el_lo  (gathered bf16 cols 257, 258)
                    nc.vector.tensor_tensor(
                        out=t_sb[:, :, None], in0=t_sb[:, :, None],
                        in1=ghr[:, :, 257:258], op=mybir.AluOpType.add,
                    )
                    nc.vector.tensor_tensor(
                        out=t_sb[:, :, None], in0=t_sb[:, :, None],
                        in1=ghr[:, :, 258:259], op=mybir.AluOpType.add,
                    )
                    t2_sb = small_p.tile([128, CH], f32, tag="t2")
                    nc.vector.tensor_scalar_mul(t2_sb[:], t_sb[:], NEG_SLOPE)
                    lk_sb = small_p.tile([128, CH], f32, tag="lk")
                    nc.vector.tensor_tensor(
                        out=lk_sb[:], in0=t_sb[:], in1=t2_sb[:],
                        op=mybir.AluOpType.max,
                    )
                    wv_sb = small_p.tile([128, CH], f32, tag="wv")
                    nc.scalar.activation(
                        wv_sb[:], lk_sb[:], mybir.ActivationFunctionType.Exp
                    )

                    selw = selw_p.tile([128, CH, 128], bf16, tag="selw")
                    nc.vector.tensor_tensor(
                        out=selw[:],
                        in0=sel[:],
                        in1=wv_sb[:, :, None].broadcast_to([128, CH, 128]),
                        op=mybir.AluOpType.mult,
                    )
                    psum_u = psu_p.tile([128, D + 1], f32, tag="pu")
                    for c in range(CH):
                        nc.tensor.matmul(
                            psum_u[:], lhsT=selw[:, c, :], rhs=ghr[:, c, 0:D + 1],
                            start=(c == 0), stop=(c == CH - 1),
                        )

                    # out = u / s + bias  (s==0 only on discarded pad slots)
                    rcp = small_p.tile([128, 1], f32, tag="rcp")
                    nc.vector.reciprocal(rcp[:], psum_u[:, D:D + 1])
                    ot = out_p.tile([128, D], f32, tag="ot")
                    nc.vector.tensor_scalar_mul(ot[:], psum_u[:, 0:D], rcp[:])
                    nc.vector.tensor_tensor(
                        out=ot[:], in0=ot[:], in1=bias_rep[:],
                        op=mybir.AluOpType.add,
                    )
                    nc.sync.dma_start(out_ext[w * 128:(w + 1) * 128, :], ot[:])

    nc.compile()
    return nc


def _get_program(ncores):
    if ncores not in _prog_cache:
        _prog_cache[ncores] = _build_program(ncores)
    return _prog_cache[ncores]


def kernel(feats, W, attn_l, attn_r, bias, src, dst):
    from concourse.bass_utils import run_bass_kernel_spmd

    feats = np.asarray(feats, np.float32)
    W = np.asarray(W, np.float32)
    attn_l = np.asarray(attn_l, np.float32)
    attn_r = np.asarray(attn_r, np.float32)
    bias = np.asarray(bias, np.float32)
    src = np.asarray(src)
    dst = np.asarray(dst)

    in_maps = _prep_inputs(feats, W, attn_l, attn_r, bias, src, dst)
    nc = _get_program(NCORES)
    res = run_bass_kernel_spmd(nc, in_maps, list(range(NCORES)))
    shards = [np.asarray(res.results[c]["out"]) for c in range(NCORES)]
    return np.concatenate(shards, axis=0)[:N].astype(np.float32)
